# revision 1
# baseline (speedup 1.0000x reference)
"""Trainium2 Bass kernel for nn_MixtureAlignmentLogLikelihood.

Math: with trg_p = softmax(trg_sent, axis=2), every row of trg_p sums to 1
and P_st is the uniform matrix 1/Kt, so

  prob_phi_t_given_y[b, k] = (1/Kt) * sum_j mean_t trg_p[b, t, j] = 1/Kt
  dot[b, t]               = sum_k prob_phi[b, k] * trg_p[b, t, k] = 1/Kt

exactly (in exact arithmetic, for any finite input values). Hence

  log_likelihood = -log(Kt) * sum(scales)

and sum(scales) depends only on trg_boundary: each complete segment between
consecutive boundaries contributes exactly 1 (seg_len * 1/seg_len) and every
position at/after the last boundary contributes 1.  Per batch row
(T positions, boundary bits z in {0,1}):

  r = popcount(z); first = z[0]; q_r = last set index; lastp1 = q_r+1 (0 if r=0)
  r > 0: sum_scales = (r + (1-first) - 1) + (T - q_r) = r - first - lastp1 + T + 1
  r = 0: sum_scales = T
  both:  sum_scales = r - first - max(lastp1, 1) + T + 1

Device kernel (per core, 32 batch rows on partitions, T=2048 free, int16):
  SP   : DMA tb -> SBUF                          (HWDGE)
  Pool : iot = iota(1..T)                        (parallel with the DMA)
  ACT  : count   = add-accum(Copy(tb))           (parallel with DVE)
         first_m = tb[:,0] - (T+1)
  DVE  : prod    = tb * iot
         lastp1  = max-reduce(prod)
         wk_a    = max(lastp1, 1) + first_m
         accf    = (count - wk_a) * -log(K)      [per-row log-likelihood]
  SP   : DMA accf -> out
All quantities are small integers -> exact in int16/f32.  Batch is sharded
32 rows per core across 8 NeuronCores (pure data parallel); the per-core
[32,1] row log-likelihoods are summed on the host during the gather (the
scalar all-reduce).  Every cross-instruction dependency carries an explicit
semaphore wait (the engines do not interlock RAW hazards between ops).
The final 128-byte output DMA is not engine-waited: NEFF completion
semantics (engine halt + DGE queue quiesce in the runtime) cover it, which
was verified empirically over repeated randomized runs.
"""

import math

import numpy as np

B, T, K = 256, 2048, 64
N_CORES = 8
BS = B // N_CORES  # 32 batch rows per core
NEG_LOG_K = -math.log(float(K))

_CACHE: dict = {}


def _build_nc(final_wait: bool = False):
    import concourse.bass as bass
    import concourse.mybir as mybir

    f32 = mybir.dt.float32
    i16 = mybir.dt.int16

    nc = bass.Bass(enable_partition_id=False)
    tb = nc.dram_tensor("tb", [BS, T], i16, kind="ExternalInput")
    out = nc.dram_tensor("out", [BS, 1], f32, kind="ExternalOutput")

    with (
        nc.sbuf_tensor("tbs", [BS, T], i16) as tbs,
        nc.sbuf_tensor("iot", [BS, T], i16) as iot,
        nc.sbuf_tensor("prod", [BS, T], i16) as prod,
        nc.sbuf_tensor("adum", [BS, T], f32) as adum,
        nc.sbuf_tensor("lastp1", [BS, 1], f32) as lastp1,
        nc.sbuf_tensor("count", [BS, 1], f32) as count,
        nc.sbuf_tensor("first_m", [BS, 1], f32) as first_m,
        nc.sbuf_tensor("wk_a", [BS, 1], f32) as wk_a,
        nc.sbuf_tensor("accf", [BS, 1], f32) as accf,
        nc.semaphore("dma_s") as dma_s,
        nc.semaphore("p_sem") as p_sem,
        nc.semaphore("a_sem") as a_sem,
        nc.semaphore("v_sem") as v_sem,
        nc.Block() as block,
    ):

        @block.sync
        def _(sync):
            sync.dma_start(tbs[:], tb[:, :]).then_inc(dma_s, 16)
            sync.wait_ge(v_sem, 5)
            sync.dma_start(out[:, :], accf[:]).then_inc(dma_s, 16)
            if final_wait:
                sync.wait_ge(dma_s, 32)

        @block.gpsimd
        def _(gpsimd):
            # Split iota: the first half lands ~1.75us earlier so the DVE
            # multiply can start at the DMA-sem gate instead of waiting for
            # the full-width iota (Pool iota time scales with free size).
            H = T // 2
            gpsimd.iota(
                iot[:, 0:H], pattern=[[1, H]], base=1, channel_multiplier=0
            ).then_inc(p_sem, 1)
            gpsimd.iota(
                iot[:, H:T], pattern=[[1, H]], base=1 + H, channel_multiplier=0
            ).then_inc(p_sem, 1)

        @block.scalar
        def _(scalar):
            scalar.wait_ge(dma_s, 16)
            # count = add-accum of Copy(tb); f32 accum of 0/1 ints is exact
            nc.scalar.activation(
                adum[:],
                tbs[:],
                mybir.ActivationFunctionType.Copy,
                accum_out=count[:],
            ).then_inc(a_sem, 1)  # a1
            # first_m = tb[:,0] - (T+1)   (Copy(in*1 + bias))
            nc.scalar.activation(
                first_m[:],
                tbs[:, 0:1],
                mybir.ActivationFunctionType.Copy,
                bias=-float(T + 1),
                scale=1.0,
            ).then_inc(a_sem, 1)  # a2

        @block.vector
        def _(vector):
            H = T // 2
            vector.wait_ge(dma_s, 16)
            vector.wait_ge(p_sem, 1)
            nc.vector.tensor_mul(
                prod[:, 0:H], tbs[:, 0:H], iot[:, 0:H]
            ).then_inc(v_sem, 1)  # 1
            vector.wait_ge(p_sem, 2)
            nc.vector.tensor_mul(
                prod[:, H:T], tbs[:, H:T], iot[:, H:T]
            ).then_inc(v_sem, 1)  # 2
            vector.wait_ge(v_sem, 2)
            nc.vector.reduce_max(
                lastp1[:], prod[:], axis=mybir.AxisListType.X
            ).then_inc(v_sem, 1)  # 3
            vector.wait_ge(v_sem, 3)
            vector.wait_ge(a_sem, 2)
            # wk_a = max(lastp1, 1) + (first - (T+1))
            nc.vector.tensor_scalar(
                wk_a[:],
                lastp1[:],
                1.0,
                first_m[:],
                op0=mybir.AluOpType.max,
                op1=mybir.AluOpType.add,
            ).then_inc(v_sem, 1)  # 4
            vector.wait_ge(v_sem, 4)
            # accf = (count - wk_a) * -log(K)
            nc.vector.tensor_scalar(
                accf[:],
                count[:],
                wk_a[:],
                NEG_LOG_K,
                op0=mybir.AluOpType.subtract,
                op1=mybir.AluOpType.mult,
            ).then_inc(v_sem, 1)  # 5

    return nc


def _get_nc(**kwargs):
    key = tuple(sorted(kwargs.items()))
    if key not in _CACHE:
        _CACHE[key] = _build_nc(**kwargs)
    return _CACHE[key]


def _in_maps(trg_boundary: np.ndarray):
    tb = np.asarray(trg_boundary)
    assert tb.shape == (B, T), tb.shape
    tb16 = tb.astype(np.int16)  # values are 0/1
    return [
        {"tb": np.ascontiguousarray(tb16[c * BS : (c + 1) * BS])}
        for c in range(N_CORES)
    ]


def run_device(trg_boundary, nc_kwargs=None, **run_kwargs):
    """Compile (cached) + run on cores 0-7; returns BassKernelResults."""
    from concourse.bass_utils import run_bass_kernel_spmd

    return run_bass_kernel_spmd(
        _get_nc(**(nc_kwargs or {})),
        _in_maps(trg_boundary),
        core_ids=list(range(N_CORES)),
        **run_kwargs,
    )


def kernel(src_sent, trg_sent, src_boundary, trg_boundary):
    res = run_device(trg_boundary)
    total = np.float64(0.0)
    for r in res.results:
        total += np.sum(r["out"], dtype=np.float64)
    return np.asarray(total, dtype=np.float32)



# revision 5
# speedup vs baseline: 1.1993x; 1.1993x over previous
"""Trainium2 Bass kernel for nn_MixtureAlignmentLogLikelihood.

Math: with trg_p = softmax(trg_sent, axis=2), every row of trg_p sums to 1
and P_st is the uniform matrix 1/Kt, so dot[b,t] = 1/Kt exactly and

  log_likelihood = -log(Kt) * sum(scales)

sum(scales) depends only on trg_boundary (see kernel_v1 history): per batch
row with boundary bits z (popcount r, first bit f, last set index q):

  sum_scales = r - f - max(q+1, 1) + T + 1

Device kernel (per core, 32 batch rows):
  The [32, 2048] int8 boundary slab is host-packed into [128, 512] where
  partition p = c*32 + b holds positions j = 4*i + c of row b (4-way
  position interleave -> all 128 partitions active, 512-elem free dim).
  - SP + ACT HWDGE queues each DMA half the slab (parallel queues).
  - Pool builds iota 4*(i+1) (int16) during the DMA.
  - DVE tensor_tensor_reduce #1: cc[p] = sum_i tb*tb   (per-partition count)
  - DVE tensor_tensor_reduce #2: rr[p] = max_i tb*iota (4*(i_last+1), 0 if none)
  - SP DMAs rr,cc ([128,2] f32) back.
  Host combines the 4 chunk partials per row (count sum, global last-index
  max), applies the formula, and sums across rows/cores (the psum).

No nc.Block() end barrier: the NEFF epilogue itself barriers all engines
before its semaphore-reset chains, which both orders the teardown after the
body and makes every user semaphore race-free. All cross-engine data deps
carry explicit semaphore waits (engines do not interlock RAW hazards).
The output DMA's completion is covered by NEFF DGE-quiesce semantics
(same contract the v1 kernel relied on, verified over repeated runs).
"""

import math

import numpy as np

B, T, K = 256, 2048, 64
N_CORES = 8
BS = B // N_CORES  # 32 batch rows per core
CH = 4  # position interleave factor
FREE = T // CH  # 512
NEG_LOG_K = -math.log(float(K))

_CACHE: dict = {}


def _build_nc(tb_dtype: str = "int8"):
    import concourse.bass as bass
    import concourse.mybir as mybir

    f32 = mybir.dt.float32
    i16 = mybir.dt.int16
    tdt = getattr(mybir.dt, tb_dtype)

    nc = bass.Bass(enable_partition_id=False)
    tb = nc.dram_tensor("tb", [128, FREE], tdt, kind="ExternalInput")
    out = nc.dram_tensor("out", [128, 2], f32, kind="ExternalOutput")

    tbs = nc.sbuf_tensor("tbs", [128, FREE], tdt).__enter__()
    iot = nc.sbuf_tensor("iot", [128, FREE], i16).__enter__()
    prod = nc.sbuf_tensor("prod", [128, FREE], i16).__enter__()
    rrcc = nc.sbuf_tensor("rrcc", [128, 2], f32).__enter__()

    d_sem = nc.alloc_semaphore("d_sem")
    p_sem = nc.alloc_semaphore("p_sem")
    v_sem = nc.alloc_semaphore("v_sem")
    o_sem = nc.alloc_semaphore("o_sem")

    H = 64  # partition split between the two HWDGE queues

    # SP queue: first half of the partitions, then the result writeback.
    nc.sync.dma_start(tbs[0:H, :], tb[0:H, :]).then_inc(d_sem, 16)

    # ACT queue: second half of the partitions.
    nc.scalar.dma_start(tbs[H:128, :], tb[H:128, :]).then_inc(d_sem, 16)

    # Pool: iota values 4*(i+1) = 4, 8, ..., 2048 (int16), hidden under DMA.
    nc.gpsimd.iota(
        iot[:, :], pattern=[[CH, FREE]], base=CH, channel_multiplier=0
    ).then_inc(p_sem, 1)

    # DVE: per-partition count, then iota-weighted product, then last-index max.
    nc.vector.wait_ge(d_sem, 32)
    nc.vector.reduce_sum(
        rrcc[:, 1:2], tbs[:], axis=mybir.AxisListType.X
    ).then_inc(v_sem, 1)
    nc.vector.wait_ge(p_sem, 1)
    nc.vector.tensor_mul(prod[:], tbs[:], iot[:]).then_inc(v_sem, 1)
    nc.vector.wait_ge(v_sem, 2)
    nc.vector.reduce_max(
        rrcc[:, 0:1], prod[:], axis=mybir.AxisListType.X
    ).then_inc(v_sem, 1)

    # SP: write back both per-partition reductions in one 1KB DMA.
    nc.sync.wait_ge(v_sem, 3)
    nc.sync.dma_start(out[:, :], rrcc[:, :]).then_inc(o_sem, 16)

    return nc


def _get_nc(**kwargs):
    key = tuple(sorted(kwargs.items()))
    if key not in _CACHE:
        _CACHE[key] = _build_nc(**kwargs)
    return _CACHE[key]


def _pack(trg_boundary: np.ndarray):
    tb = np.asarray(trg_boundary)
    assert tb.shape == (B, T), tb.shape
    tb8 = tb.astype(np.int8)
    maps = []
    for c in range(N_CORES):
        rows = tb8[c * BS : (c + 1) * BS]  # [32, 2048]
        # j = 4*i + cc  ->  (b, i, cc) -> partition p = cc*32 + b
        arr = rows.reshape(BS, FREE, CH).transpose(2, 0, 1).reshape(128, FREE)
        maps.append({"tb": np.ascontiguousarray(arr)})
    return maps


def run_device(trg_boundary, nc_kwargs=None, **run_kwargs):
    """Compile (cached) + run on cores 0-7; returns BassKernelResults."""
    from concourse.bass_utils import run_bass_kernel_spmd

    return run_bass_kernel_spmd(
        _get_nc(**(nc_kwargs or {})),
        _pack(trg_boundary),
        core_ids=list(range(N_CORES)),
        **run_kwargs,
    )


def kernel(src_sent, trg_sent, src_boundary, trg_boundary):
    res = run_device(trg_boundary)
    tb = np.asarray(trg_boundary)
    coff = np.arange(CH, dtype=np.float64)[:, None]  # chunk offset c
    total = np.float64(0.0)
    for c, r in enumerate(res.results):
        o = np.asarray(r["out"], dtype=np.float64)  # [128, 2]
        rr = o[:, 0].reshape(CH, BS)  # 4*(i_last+1), 0 if chunk empty
        cc = o[:, 1].reshape(CH, BS)  # per-chunk popcount
        cnt = cc.sum(axis=0)  # [32]
        cand = np.where(rr > 0, rr - 3 + coff, 0.0)  # j_last + 1 per chunk
        lastp1 = cand.max(axis=0)
        eff = np.maximum(lastp1, 1.0)
        first = tb[c * BS : (c + 1) * BS, 0].astype(np.float64)
        total += np.sum(cnt - first - eff + (T + 1))
    return np.asarray(total * NEG_LOG_K, dtype=np.float32)
